# revision 64
# baseline (speedup 1.0000x reference)
"""Trainium2 Bass kernel for nn_LuminaLM (4-layer GPT-2-like transformer + LM head).

Strategy: 8-way Megatron tensor parallel with sequence-parallel residual.
 - Each core owns 2 of 16 heads, 1/8 of the vocab; MLP is token-local.
 - Residual h is token-sharded: core r owns tokens [128r,128r+128) of each batch,
   stored feature-major as [128(dp), 8(dt), 128(t)] fp32 in SBUF.
 - All LN gains/biases are folded into the consuming weights/biases on the HOST,
   so on-device LN is a pure (h-m)*rstd normalize. LN statistics are computed
   INCREMENTALLY: the per-d-tile sum / sum-of-squares matmuls are emitted inside
   the producer loops (fc2 residual adds, RS residual adds, embedding), so only
   the tiny finish chain sits on the critical path.
 - Weights are staged host-side in bf16 with partition-contiguous layouts.
 - Per layer: LN1 both halves -> one combined AllGather -> per half
   qkv -> attention -> proj partial -> ReduceScatter -> residual -> LN2;
   MLP (fc1+gelu+fc2) runs jointly over both halves (256 tokens).
 - Attention computes S^T = k^T q directly (keys on partitions), exp on ScalarE
   PSUM->SBUF, causal masking via affine_select on the diagonal tiles, and AV
   with a ones-augmented v so the softmax denominator falls out of the same
   matmul; normalization happens on PSUM eviction with a PE-broadcast
   reciprocal. No per-query-tile cross-engine round trips.
 - LM head: vocab-sharded, bf16 weights streamed, logits fp32 out.
Matmuls are bf16 with fp32 PSUM accumulation; collectives ride bf16.
"""

import os
import numpy as np

B, T, D, V, L = 2, 1024, 1024, 32000, 4
H, HD = 16, 64
NCORES = 8
P = 128
TPC = T // NCORES          # 128 tokens per core per batch
HPC = H // NCORES          # 2 heads per core
QKVC = 3 * P               # 384 qkv cols per core (q:128, k:128, v:128)
VPC = V // NCORES          # 4000 vocab per core
MC = 125                   # lm-head M chunk (32 chunks of 125 = 4000)
NMC = VPC // MC            # 32
DT = D // P                # 8 d-tiles
NFC = 4 * D // P           # 32 fc1-output chunks
EPS = 1e-5
ATT_SCALE = 1.0 / np.sqrt(HD)

_CACHE = {}
last_exec_time_ns = None


def _build_nc(no_coll=False):
    import concourse.bass as bass
    import concourse.mybir as mybir
    import concourse.tile as tile
    from concourse import bacc
    from concourse.masks import make_identity
    from concourse.bass import IndirectOffsetOnAxis

    dt = mybir.dt
    AF = mybir.ActivationFunctionType
    OP = mybir.AluOpType

    nc = bacc.Bacc("TRN2", target_bir_lowering=False, debug=False,
                   num_devices=NCORES)

    # ---- external parameters (per-core shards, staged by host) ----
    ids_p = nc.declare_dram_parameter("ids", [B, TPC], dt.int32, isOutput=False)
    wte_p = nc.declare_dram_parameter("wte", [V, D], dt.float32, isOutput=False)
    wpe_p = nc.declare_dram_parameter("wpe_sh", [TPC, D], dt.float32, isOutput=False)
    wqkv_p = nc.declare_dram_parameter("wqkv_sh", [L, P, DT, QKVC], dt.bfloat16, isOutput=False)
    bqkv_p = nc.declare_dram_parameter("bqkv_sh", [L, P, 3], dt.float32, isOutput=False)
    wproj_p = nc.declare_dram_parameter("wproj_sh", [L, P, D], dt.bfloat16, isOutput=False)
    bproj_p = nc.declare_dram_parameter("bproj", [L, D], dt.float32, isOutput=False)
    wfc1_p = nc.declare_dram_parameter("wfc1_st", [L, NFC, P, DT, P], dt.bfloat16, isOutput=False)
    bfc1_p = nc.declare_dram_parameter("bfc1_st", [L, P, NFC], dt.float32, isOutput=False)
    wfc2_p = nc.declare_dram_parameter("wfc2_st", [L, DT, P, NFC, P], dt.bfloat16, isOutput=False)
    bfc2_p = nc.declare_dram_parameter("bfc2", [L, D], dt.float32, isOutput=False)
    wlm_p = nc.declare_dram_parameter("wlm_st", [NMC, P, DT, MC], dt.bfloat16, isOutput=False)
    blm_p = nc.declare_dram_parameter("blm_st", [MC, NMC], dt.float32, isOutput=False)
    logits_p = nc.declare_dram_parameter("logits", [VPC, B * T], dt.float32, isOutput=True)

    RG = [list(range(NCORES))]
    HD1 = HD + 1

    with tile.TileContext(nc) as tc:
        with (
            tc.tile_pool(name="const", bufs=1) as cp,
            tc.tile_pool(name="wp", bufs=2) as wp,
            tc.tile_pool(name="ap", bufs=2) as app,
            tc.tile_pool(name="psA", bufs=3, space="PSUM") as psA,
            tc.tile_pool(name="psS", bufs=3, space="PSUM") as psS,
            tc.tile_pool(name="psB", bufs=2, space="PSUM") as psB,
            tc.tile_pool(name="dram", bufs=2, space="DRAM") as dramp,
        ):
            # ---------------- warmup collective ----------------
            warm_in = dramp.tile([P, 2], dt.bfloat16, name="warm_in", tag="wrm")
            warm_out = dramp.tile([NCORES * P, 2], dt.bfloat16, name="warm_out",
                                  tag="wrmo", addr_space="Shared")
            warm_sb = cp.tile([P, 2], dt.bfloat16, name="warm_sb")
            nc.any.memset(warm_sb[:], 0.0)
            nc.sync.dma_start(warm_in[:], warm_sb[:])
            if no_coll:
                nc.sync.dma_start(warm_out[0:P, :], warm_in[:])
            else:
                nc.gpsimd.collective_compute(
                    "AllGather", OP.bypass, replica_groups=RG,
                    ins=[warm_in[:].opt()], outs=[warm_out[:].opt()],
                )

            # ---------------- constants ----------------
            ident_bf = cp.tile([P, P], dt.bfloat16)
            make_identity(nc, ident_bf[:])
            ident_f = cp.tile([P, P], dt.float32)
            make_identity(nc, ident_f[:])
            onesD_bf = cp.tile([P, 1], dt.bfloat16)
            nc.any.memset(onesD_bf[:], 1.0 / D)
            onesD_f = cp.tile([P, 1], dt.float32)
            nc.any.memset(onesD_f[:], 1.0 / D)
            ones_row_f = cp.tile([1, P], dt.float32)
            nc.any.memset(ones_row_f[:], 1.0)
            ones_row_bf = cp.tile([1, P], dt.bfloat16)
            nc.any.memset(ones_row_bf[:], 1.0)
            eps_t = cp.tile([1, 1], dt.float32)
            nc.any.memset(eps_t[:], EPS)

            # per-layer bias tiles (fp32, tiny)
            bqkvt = []
            bfc1t = []
            bprojt = []
            bfc2t = []
            for li in range(L):
                t = cp.tile([P, 3], dt.float32, name=f"bqkv{li}")
                nc.sync.dma_start(t[:], bqkv_p[li])
                bqkvt.append(t)
                t = cp.tile([P, NFC], dt.float32, name=f"bfc1{li}")
                nc.sync.dma_start(t[:], bfc1_p[li])
                bfc1t.append(t)
                t = cp.tile([P, DT], dt.float32, name=f"bproj{li}")
                nc.sync.dma_start(t[:], bproj_p[li].rearrange("(c p) -> p c", p=P))
                bprojt.append(t)
                t = cp.tile([P, DT], dt.float32, name=f"bfc2{li}")
                nc.sync.dma_start(t[:], bfc2_p[li].rearrange("(c p) -> p c", p=P))
                bfc2t.append(t)
            blm_all = cp.tile([MC, NMC], dt.float32, name="blm_all")
            nc.sync.dma_start(blm_all[:], blm_p[:])

            # wpe [128 tok, D]
            wpe_tok = cp.tile([TPC, D], dt.float32)
            nc.sync.dma_start(wpe_tok[:], wpe_p[:])
            # token indices [128, B] int32
            idx_sb = cp.tile([TPC, B], dt.int32)
            nc.sync.dma_start(idx_sb[:], ids_p[:].rearrange("b t -> t b"))

            # ---------------- LN helpers (incremental stats) ----------------
            def ln_alloc(name, width):
                ps_sum = psB.tile([1, width], dt.float32, space="PSUM",
                                  name=f"psum_{name}", tag="psB")
                ps_sq = psB.tile([1, width], dt.float32, space="PSUM",
                                 name=f"psq_{name}", tag="psB")
                return ps_sum, ps_sq

            def ln_stats_all(stats, h_tile, off, first, last, name):
                """Sum/sumsq of h_tile into stats column range [off, off+TPC)
                (1/D folded into the ones vectors). Emitted as consecutive
                matmuls to avoid thrashing the PE weight pipeline. `first`
                must be True for exactly ONE matmul per stats tile (start
                clears the whole PSUM bank's has-written bits)."""
                ps_sum, ps_sq = stats
                hb2 = app.tile([P, DT, TPC], dt.bfloat16, name=f"hb2_{name}", tag="hb2")
                nc.scalar.activation(hb2[:], h_tile[:], AF.Square)
                for dc in range(DT):
                    nc.tensor.matmul(ps_sum[:, off:off + TPC], lhsT=onesD_f[:],
                                     rhs=h_tile[:, dc, :],
                                     start=(first and dc == 0),
                                     stop=(last and dc == DT - 1),
                                     skip_group_check=True)
                for dc in range(DT):
                    nc.tensor.matmul(ps_sq[:, off:off + TPC], lhsT=onesD_bf[:],
                                     rhs=hb2[:, dc, :],
                                     start=(first and dc == 0),
                                     stop=(last and dc == DT - 1),
                                     skip_group_check=True)

            def ln_finish(stats, off, pool, ptag, name):
                """Small finish chain -> broadcast (rstd_full, mrstd_full)."""
                ps_sum, ps_sq = stats
                mm_sb = app.tile([1, TPC], dt.float32, name=f"mm_{name}", tag="mm")
                nc.scalar.activation(mm_sb[:], ps_sum[:, off:off + TPC], AF.Square)
                var_sb = app.tile([1, TPC], dt.float32, name=f"var_{name}", tag="var")
                nc.vector.scalar_tensor_tensor(
                    out=var_sb[:], in0=ps_sq[:, off:off + TPC], scalar=1.0,
                    in1=mm_sb[:], op0=OP.mult, op1=OP.subtract)
                std_sb = app.tile([1, TPC], dt.float32, name=f"std_{name}", tag="std")
                nc.scalar.activation(std_sb[:], var_sb[:], AF.Sqrt, bias=eps_t[:])
                rstd_sb = app.tile([1, TPC], dt.float32, name=f"rstd_{name}", tag="rstd")
                nc.vector.reciprocal_approx_fast(rstd_sb[:], std_sb[:])
                mrstd_sb = app.tile([1, TPC], dt.float32, name=f"mrstd_{name}", tag="mrstd")
                nc.vector.scalar_tensor_tensor(
                    out=mrstd_sb[:], in0=ps_sum[:, off:off + TPC], scalar=-1.0,
                    in1=rstd_sb[:], op0=OP.mult, op1=OP.mult)
                ps_r = pool.tile([P, TPC], dt.float32, space="PSUM",
                                 name=f"psr_{name}", tag=ptag)
                nc.tensor.matmul(ps_r[:], lhsT=ones_row_f[:], rhs=rstd_sb[:],
                                 start=True, stop=True)
                rstd_full = app.tile([P, TPC], dt.float32, name=f"rstdf_{name}", tag="rstdf")
                nc.vector.tensor_copy(rstd_full[:], ps_r[:])
                ps_mr = pool.tile([P, TPC], dt.float32, space="PSUM",
                                  name=f"psmr_{name}", tag=ptag)
                nc.tensor.matmul(ps_mr[:], lhsT=ones_row_f[:], rhs=mrstd_sb[:],
                                 start=True, stop=True)
                mrstd_full = app.tile([P, TPC], dt.float32, name=f"mrstdf_{name}", tag="mrstdf")
                nc.vector.tensor_copy(mrstd_full[:], ps_mr[:])
                return rstd_full, mrstd_full

            def normalize(h_tile, out_ap, rstd_full, mrstd_full, name):
                t1 = app.tile([P, DT, TPC], dt.bfloat16, name=f"t1_{name}", tag="t1n")
                nc.vector.tensor_tensor(
                    out=t1[:], in0=h_tile[:],
                    in1=rstd_full[:, None, :].to_broadcast([P, DT, TPC]), op=OP.mult)
                nc.vector.tensor_tensor(
                    out=out_ap, in0=t1[:],
                    in1=mrstd_full[:, None, :].to_broadcast([P, DT, TPC]), op=OP.add)

            # ---------------- embedding ----------------
            hres = [cp.tile([P, DT, TPC], dt.float32, name=f"hres{h}") for h in range(B)]
            emb_stats = ln_alloc("emb", B * TPC)
            for half in range(B):
                emb = app.tile([TPC, D], dt.float32, name="emb", tag="emb", bufs=1)
                nc.gpsimd.indirect_dma_start(
                    out=emb[:], out_offset=None, in_=wte_p[:],
                    in_offset=IndirectOffsetOnAxis(ap=idx_sb[:, half:half + 1], axis=0),
                )
                nc.vector.tensor_add(emb[:], emb[:], wpe_tok[:])
                for dti in range(DT):
                    pst = psA.tile([P, P], dt.float32, space="PSUM", name="pst_emb",
                                   tag="psA")
                    nc.tensor.transpose(pst[:], emb[:, dti * P:(dti + 1) * P], ident_f[:])
                    nc.vector.tensor_copy(hres[half][:, dti, :], pst[:])
                ln_stats_all(emb_stats, hres[half], half * TPC,
                             first=(half == 0), last=(half == B - 1),
                             name=f"e{half}")

            # ---------------- AG / RS ----------------
            def allgather_read(name, stats):
                """Finish LN1 for both halves, bounce, combined AllGather,
                read back per-half aT tiles (h0 first so qkv0 starts asap)."""
                ag_in = dramp.tile([B * D, TPC], dt.bfloat16, name=f"agin_{name}",
                                   tag="agin")
                for h in range(B):
                    rstd_full, mrstd_full = ln_finish(stats, h * TPC, psS, "psS",
                                                      f"{name}h{h}")
                    hn = app.tile([P, DT, TPC], dt.bfloat16, name=f"hn_{name}{h}",
                                  tag="hn")
                    normalize(hres[h], hn[:], rstd_full, mrstd_full, f"{name}h{h}")
                    nc.sync.dma_start(
                        ag_in[h * D:(h + 1) * D, :].rearrange(
                            "(dt p) t -> p dt t", p=P), hn[:])
                ag_out = dramp.tile([NCORES * B * D, TPC], dt.bfloat16,
                                    name=f"agout_{name}", tag="agout",
                                    addr_space="Shared")
                if no_coll:
                    nc.sync.dma_start(ag_out[0:B * D, :], ag_in[:])
                else:
                    nc.gpsimd.collective_compute(
                        "AllGather", OP.bypass, replica_groups=RG,
                        ins=[ag_in[:].opt()], outs=[ag_out[:].opt()],
                    )
                ag_view = ag_out[:].rearrange("(r b dt p) t -> b p dt r t",
                                              p=P, dt=DT, b=B)
                aTs = []
                for h in range(B):
                    aT = app.tile([P, DT, NCORES, TPC], dt.bfloat16,
                                  name=f"aT_{name}{h}", tag="aT")
                    for dti in range(DT):
                        nc.sync.dma_start(aT[:, dti, :, :], ag_view[h, :, dti, :, :])
                    aTs.append(aT.rearrange("p dt r t -> p dt (r t)"))
                return aTs

            def reduce_scatter_residual(rs_in, bias_t, h_tile, half, name):
                """RS by token block, add into residual, accumulate LN2 stats."""
                rs_out = dramp.tile([D, TPC], dt.bfloat16, name=f"rsout_{name}", tag="rsout")
                if no_coll:
                    nc.sync.dma_start(rs_out[:], rs_in[0:D, :])
                else:
                    nc.gpsimd.collective_compute(
                        "ReduceScatter", OP.add, replica_groups=RG,
                        ins=[rs_in[:].opt()], outs=[rs_out[:].opt()],
                    )
                rsb = app.tile([P, DT, TPC], dt.bfloat16, name=f"rsb_{name}", tag="rsb")
                nc.sync.dma_start(rsb[:], rs_out[:].rearrange("(dc p) t -> p dc t", p=P))
                stats = ln_alloc(name, TPC)
                for dc in range(DT):
                    nc.vector.scalar_tensor_tensor(
                        out=h_tile[:, dc, :], in0=rsb[:, dc, :],
                        scalar=bias_t[:, dc:dc + 1], in1=h_tile[:, dc, :],
                        op0=OP.add, op1=OP.add)
                ln_stats_all(stats, h_tile, 0, first=True, last=True, name=name)
                return stats

            def load_weights(li):
                wqkv = wp.tile([P, DT, QKVC], dt.bfloat16, name=f"wqkv{li}", tag="wqkv")
                nc.sync.dma_start(wqkv[:], wqkv_p[li])
                wproj = wp.tile([P, D], dt.bfloat16, name=f"wproj{li}", tag="wproj")
                nc.sync.dma_start(wproj[:], wproj_p[li])
                return wqkv, wproj

            NT = T // 512  # 2 token chunks of 512 per half

            def qkv_block(aT, wqkv, bqkv, half):
                qkvT = app.tile([P, 3, T], dt.bfloat16, name=f"qkvT{half}", tag="qkvT")
                for c in range(3):
                    for tk in range(NT):
                        ps = psA.tile([P, 512], dt.float32, space="PSUM", name="ps_qkv", tag="psA")
                        for dti in range(DT):
                            nc.tensor.matmul(
                                ps[:], lhsT=wqkv[:, dti, c * P:(c + 1) * P],
                                rhs=aT[:, dti, tk * 512:(tk + 1) * 512],
                                start=(dti == 0), stop=(dti == DT - 1))
                        nc.vector.tensor_scalar_add(
                            qkvT[:, c, tk * 512:(tk + 1) * 512], ps[:],
                            bqkv[:, c:c + 1])
                return qkvT

            def attention_both(qkvTs, wproj, nprefix):
                """Both halves' attention, software-pipelined at group
                granularity: S^T/exp of group n+1 is emitted BEFORE the AV of
                group n, so the PE always has independent matmuls in its
                in-order stream while ScalarE works through the exps.
                S^T = k^T q per key-tile, exp PSUM->SBUF, affine-select causal
                mask, AV with ones-augmented v; normalization on eviction."""
                v_augs = []
                for half in range(B):
                    v_aug = app.tile([P, DT, 2 * HD1], dt.bfloat16,
                                     name=f"vaug{half}", tag="vaug")
                    for h2 in range(HPC):
                        nc.any.memset(
                            v_aug[:, :, h2 * HD1 + HD:h2 * HD1 + HD1], 1.0)
                    for tt in range(DT):
                        pst = psB.tile([P, P], dt.bfloat16, space="PSUM",
                                       name="pst_v", tag="psB")
                        nc.tensor.transpose(
                            pst[:], qkvTs[half][:, 2, tt * P:(tt + 1) * P],
                            ident_bf[:])
                        for h2 in range(HPC):
                            nc.vector.tensor_copy(
                                v_aug[:, tt, h2 * HD1:h2 * HD1 + HD],
                                pst[:, h2 * HD:(h2 + 1) * HD])
                    v_augs.append(v_aug)

                yTs = [app.tile([P, T], dt.bfloat16, name=f"yT{half}", tag="yT")
                       for half in range(B)]

                tails = []
                done = [0] * B
                prs = [None] * B

                def emit_tail(ps_y, half, hs, qc):
                    den = app.tile([1, 512], dt.bfloat16, name="den", tag="den")
                    nc.vector.tensor_copy(den[:], ps_y[HD:HD1, :])
                    ps_bc = psS.tile([HD, 512], dt.float32, space="PSUM",
                                     name="ps_bc", tag="psS")
                    nc.tensor.matmul(ps_bc[:], lhsT=ones_row_bf[:, :HD], rhs=den[:],
                                     start=True, stop=True)
                    recb = app.tile([HD, 512], dt.float32, name="recb", tag="recb")
                    nc.vector.reciprocal_approx_fast(recb[:], ps_bc[:])
                    nc.vector.tensor_tensor(
                        out=yTs[half][hs:hs + HD, qc * 512:(qc + 1) * 512],
                        in0=ps_y[:HD, :], in1=recb[:], op=OP.mult)
                    done[half] += 1
                    if half == 0 and done[0] == NT * HPC:
                        # h0's yT is complete while h1 still drains exps: emit
                        # proj0 now so RS0 (which gates RS1 via the CC FIFO)
                        # fires earlier
                        prs[0] = proj_partial(yTs[0], wproj, 0, f"{nprefix}p0")

                def emit_av(PTt, half, h2, qc, nkt):
                    ps_y = psA.tile([HD1, 512], dt.float32, space="PSUM",
                                    name="ps_y", tag="psA")
                    for kt in range(nkt):
                        qlo = max(0, kt * P - qc * 512)
                        nc.tensor.matmul(
                            ps_y[:, qlo:512],
                            lhsT=v_augs[half][:, kt, h2 * HD1:(h2 + 1) * HD1],
                            rhs=PTt[:, kt, qlo:512],
                            start=(kt == 0), stop=(kt == nkt - 1))
                    tails.append((ps_y, half, h2 * HD, qc))
                    if len(tails) >= 2:
                        emit_tail(*tails.pop(0))

                # qc0 alternates halves for cross-half ILP; qc1 puts h0's two
                # groups first so h0 finishes two groups early (inline proj0)
                order = [(0, h2, half) for h2 in range(HPC) for half in range(B)]
                order += [(qc, h2, half) for qc in range(1, NT)
                          for half in range(B) for h2 in range(HPC)]
                prev = None
                for qc, h2, half in order:
                    if True:
                        if True:
                            qkvT = qkvTs[half]
                            hs = h2 * HD
                            nkt = qc * 4 + 4
                            PTt = app.tile([P, 8, 512], dt.bfloat16,
                                           name=f"PT{half}_{qc}_{h2}", tag="PT",
                                           bufs=3)
                            for kt in range(nkt):
                                qlo = max(0, kt * P - qc * 512)
                                ps_st = psS.tile([P, 512], dt.float32,
                                                 space="PSUM", name="ps_st",
                                                 tag="psS")
                                nc.tensor.matmul(
                                    ps_st[:, qlo:512],
                                    lhsT=qkvT[hs:hs + HD, 1, kt * P:(kt + 1) * P],
                                    rhs=qkvT[hs:hs + HD, 0,
                                             qc * 512 + qlo:(qc + 1) * 512],
                                    start=True, stop=True)
                                nc.scalar.activation(
                                    PTt[:, kt, qlo:512], ps_st[:, qlo:512],
                                    AF.Exp, scale=ATT_SCALE)
                                if kt >= qc * 4:
                                    nc.gpsimd.affine_select(
                                        out=PTt[:, kt, qlo:qlo + P],
                                        in_=PTt[:, kt, qlo:qlo + P],
                                        compare_op=OP.is_ge, fill=0.0, base=0,
                                        pattern=[[1, P]], channel_multiplier=-1)
                            if prev is not None:
                                emit_av(*prev)
                            prev = (PTt, half, h2, qc, nkt)
                emit_av(*prev)
                while tails:
                    emit_tail(*tails.pop(0))
                return yTs, prs

            def proj_partial(yT, wproj, half, name):
                rs_in = dramp.tile([NCORES * D, TPC], dt.bfloat16,
                                   name=f"rsin_{name}", tag="rsin")
                rs_view = rs_in[:].rearrange("(tb dc p) tw -> p dc tb tw", p=P, dc=DT)
                for dc in range(DT):
                    for tk in range(NT):
                        ps = psA.tile([P, 512], dt.float32, space="PSUM", name="ps_pr", tag="psA")
                        nc.tensor.matmul(
                            ps[:], lhsT=wproj[:, dc * P:(dc + 1) * P],
                            rhs=yT[:, tk * 512:(tk + 1) * 512], start=True, stop=True)
                        prc = app.tile([P, 512], dt.bfloat16, name="prc", tag="prc",
                                       bufs=3)
                        nc.vector.tensor_copy(prc[:], ps[:])
                        nc.sync.dma_start(
                            rs_view[:, dc, tk * 4:(tk + 1) * 4, :],
                            prc[:].rearrange("p (tb tw) -> p tb tw", tw=TPC))
                return rs_in

            def mlp(li, hn2m):
                """Token-local MLP over both halves; accumulates next-LN1 stats."""
                mTm = app.tile([P, NFC, B * TPC], dt.bfloat16, name=f"mTm{li}",
                               tag="mTm", bufs=1)
                for fc in range(NFC):
                    wf1c = wp.tile([P, DT, P], dt.bfloat16, name=f"wf1c{li}_{fc}",
                                   tag="wf1c", bufs=6)
                    nc.sync.dma_start(wf1c[:], wfc1_p[li, fc])
                    ps = psA.tile([P, 512], dt.float32, space="PSUM",
                                  name="ps_f1", tag="psA")
                    for dti in range(DT):
                        nc.tensor.matmul(
                            ps[:, :B * TPC], lhsT=wf1c[:, dti, :],
                            rhs=hn2m[:, dti, :],
                            start=(dti == 0), stop=(dti == DT - 1))
                    nc.scalar.activation(
                        mTm[:, fc, :], ps[:, :B * TPC], AF.Gelu,
                        bias=bfc1t[li][:, fc:fc + 1])
                stats = ln_alloc(f"l{li}n", B * TPC)
                for dc in range(DT):
                    wf2c = wp.tile([P, NFC, P], dt.bfloat16, name=f"wf2c{li}_{dc}",
                                   tag="wf2c", bufs=2)
                    nc.sync.dma_start(wf2c[:], wfc2_p[li, dc])
                    ps2 = psA.tile([P, 512], dt.float32, space="PSUM",
                                   name="ps_f2", tag="psA")
                    for kt in range(NFC):
                        nc.tensor.matmul(
                            ps2[:, :B * TPC], lhsT=wf2c[:, kt, :], rhs=mTm[:, kt, :],
                            start=(kt == 0), stop=(kt == NFC - 1))
                    for h in range(B):
                        nc.vector.scalar_tensor_tensor(
                            out=hres[h][:, dc, :],
                            in0=ps2[:, h * TPC:(h + 1) * TPC],
                            scalar=bfc2t[li][:, dc:dc + 1],
                            in1=hres[h][:, dc, :], op0=OP.add, op1=OP.add)
                for h in range(B):
                    ln_stats_all(stats, hres[h], h * TPC,
                                 first=(h == 0), last=(h == B - 1),
                                 name=f"l{li}n{h}")
                return stats

            # ---------------- transformer layers ----------------
            stats = emb_stats
            for li in range(L):
                wqkv, wproj = load_weights(li)
                aTs = allgather_read(f"l{li}", stats)
                qkvTs = [qkv_block(aTs[h], wqkv, bqkvt[li], h) for h in range(B)]
                yTs, prs = attention_both(qkvTs, wproj, f"l{li}")
                prs[1] = proj_partial(yTs[1], wproj, 1, f"l{li}p1")
                hn2m = app.tile([P, DT, B * TPC], dt.bfloat16, name=f"hn2m{li}",
                                tag="hn2m", bufs=2)
                for h in range(B):
                    st2 = reduce_scatter_residual(prs[h], bprojt[li], hres[h], h,
                                                  f"l{li}p{h}")
                    rstd_full, mrstd_full = ln_finish(st2, 0, psB, "psB",
                                                      f"l{li}m{h}")
                    normalize(hres[h], hn2m[:, :, h * TPC:(h + 1) * TPC],
                              rstd_full, mrstd_full, f"l{li}m{h}")
                stats = mlp(li, hn2m)

            # ---------------- final LN + LM head ----------------
            # consume the warmup AG output late (adds 0.0) so DCE keeps it
            warm_rd = cp.tile([1, 2], dt.bfloat16, name="warm_rd")
            nc.sync.dma_start(warm_rd[:], warm_out[0:1, :])
            nc.vector.tensor_add(blm_all[0:1, 0:1], blm_all[0:1, 0:1],
                                 warm_rd[0:1, 0:1])
            afTs = allgather_read("fin", stats)
            for mc in range(NMC):
                wlm = app.tile([P, DT, MC], dt.bfloat16, name=f"wlm{mc}", tag="wlm",
                               bufs=3)
                nc.sync.dma_start(wlm[:], wlm_p[mc])
                for h in range(B):
                    for tk in range(NT):
                        ps = psA.tile([P, 512], dt.float32, space="PSUM", name="ps_lm", tag="psA")
                        for dti in range(DT):
                            nc.tensor.matmul(
                                ps[:MC, :], lhsT=wlm[:, dti, :],
                                rhs=afTs[h][:, dti, tk * 512:(tk + 1) * 512],
                                start=(dti == 0), stop=(dti == DT - 1))
                        lsb = app.tile([MC, 512], dt.float32, name="lsb", tag="lsb",
                                       bufs=3)
                        nc.scalar.activation(lsb[:], ps[:MC, :], AF.Identity,
                                             bias=blm_all[:, mc:mc + 1])
                        nc.sync.dma_start(
                            logits_p[mc * MC:(mc + 1) * MC,
                                     h * T + tk * 512:h * T + (tk + 1) * 512],
                            lsb[:])

    nc.compile()
    return nc


def _get_nc():
    no_coll = os.environ.get("KERNEL_NO_COLL", "0") == "1"
    key = ("nc", no_coll)
    if key not in _CACHE:
        _CACHE[key] = _build_nc(no_coll)
    return _CACHE[key]


def build_in_maps(input_ids, wte, wpe, ln1_g, ln1_b, w_qkv, b_qkv, w_proj,
                  b_proj, ln2_g, ln2_b, w_fc1, b_fc1, w_fc2, b_fc2, lnf_g,
                  lnf_b, w_lm):
    import ml_dtypes
    f32 = np.float32
    bf16 = ml_dtypes.bfloat16

    ids = np.ascontiguousarray(np.asarray(input_ids).astype(np.int32))
    wte = np.ascontiguousarray(np.asarray(wte, dtype=f32))
    wpe = np.ascontiguousarray(np.asarray(wpe, dtype=f32))
    w_qkv = np.asarray(w_qkv, f32)
    b_qkv = np.asarray(b_qkv, f32)
    w_proj = np.asarray(w_proj, f32)
    w_fc1 = np.asarray(w_fc1, f32)
    b_fc1 = np.asarray(b_fc1, f32)
    w_fc2 = np.asarray(w_fc2, f32)
    w_lm = np.asarray(w_lm, f32)
    g1 = np.asarray(ln1_g, f32)
    b1 = np.asarray(ln1_b, f32)
    g2 = np.asarray(ln2_g, f32)
    b2 = np.asarray(ln2_b, f32)
    gf = np.asarray(lnf_g, f32)
    bf = np.asarray(lnf_b, f32)

    # Fold LN gains into consuming weights; LN biases into consuming biases.
    wqkv_fold = w_qkv * g1[:, :, None]                       # [L, D, 3D]
    bqkv_eff = b_qkv + np.einsum("ld,ldc->lc", b1, w_qkv)    # [L, 3D]
    wfc1_fold = w_fc1 * g2[:, :, None]                       # [L, D, 4D]
    bfc1_eff = b_fc1 + np.einsum("ld,ldf->lf", b2, w_fc1)    # [L, 4D]
    wlm_fold = w_lm * gf[:, None]                            # [D, V]
    blm_eff = bf @ w_lm                                      # [V]

    # Shared (core-independent) stages.
    wfc1_st = np.ascontiguousarray(
        wfc1_fold.reshape(L, DT, P, NFC, P).transpose(0, 3, 2, 1, 4).astype(bf16))
    bfc1_st = np.ascontiguousarray(bfc1_eff.reshape(L, NFC, P).transpose(0, 2, 1))
    wfc2_st = np.ascontiguousarray(
        np.asarray(w_fc2, f32).reshape(L, NFC, P, DT, P)
        .transpose(0, 3, 2, 1, 4).astype(bf16))

    in_maps = []
    for r in range(NCORES):
        t0, t1 = r * TPC, (r + 1) * TPC
        cols = np.r_[P * r:P * r + P, D + P * r:D + P * r + P,
                     2 * D + P * r:2 * D + P * r + P]
        vs, ve = r * VPC, (r + 1) * VPC
        wqkv_st = np.ascontiguousarray(
            wqkv_fold[:, :, cols].reshape(L, DT, P, QKVC)
            .transpose(0, 2, 1, 3).astype(bf16))
        bqkv_st = np.ascontiguousarray(
            bqkv_eff[:, cols].reshape(L, 3, P).transpose(0, 2, 1))
        wproj_st = np.ascontiguousarray(
            w_proj[:, P * r:P * r + P, :].astype(bf16))
        wlm_st = np.ascontiguousarray(
            wlm_fold[:, vs:ve].reshape(DT, P, NMC, MC)
            .transpose(2, 1, 0, 3).astype(bf16))
        blm_st = np.ascontiguousarray(blm_eff[vs:ve].reshape(NMC, MC).T)
        in_maps.append({
            "ids": np.ascontiguousarray(ids[:, t0:t1]),
            "wte": wte,
            "wpe_sh": np.ascontiguousarray(wpe[t0:t1]),
            "wqkv_sh": wqkv_st,
            "bqkv_sh": bqkv_st,
            "wproj_sh": wproj_st,
            "bproj": np.ascontiguousarray(np.asarray(b_proj, f32)),
            "wfc1_st": wfc1_st,
            "bfc1_st": bfc1_st,
            "wfc2_st": wfc2_st,
            "bfc2": np.ascontiguousarray(np.asarray(b_fc2, f32)),
            "wlm_st": wlm_st,
            "blm_st": blm_st,
        })

    return in_maps


def kernel(**inputs):
    global last_exec_time_ns
    from concourse.bass_utils import run_bass_kernel_spmd

    in_maps = build_in_maps(**inputs)
    nc = _get_nc()
    trace = os.environ.get("KERNEL_TRACE", "0") == "1"
    res = run_bass_kernel_spmd(nc, in_maps, list(range(NCORES)), trace=trace)
    last_exec_time_ns = res.exec_time_ns

    parts = [res.results[r]["logits"] for r in range(NCORES)]  # [VPC, B*T] each
    full = np.concatenate(parts, axis=0)          # [V, B*T]
    out = full.T.reshape(B, T, V).astype(np.float32)
    return out
